# revision 25
# baseline (speedup 1.0000x reference)
"""ChildSumTreeLSTM on 8 trn2 NeuronCores — v2 (fused).

Tree: reversed complete 4-ary heap (id = N-1-heap; heap j's children are
4j+1..4j+4).  The 64 depth-3 subtrees rooted at heap 21..84 are dealt to
cores stride-8 (subtree k -> core k%8, slot k//8) so that every core's REAL
level-6 leaves fit in its first 6 subtree slots (384 leaf columns); the
last 2 slots are always leafless, so the leaf level computes 384 columns
instead of 512.  Each core runs a uniform padded forest (levels 384-of-512
/128/32/8) and then every core redundantly computes the 21-node top tree
after a 32KB AllGather of the 64 subtree roots.

Layouts ("T layout"): mem (512 -> 4 partition chunks of 128) on partitions,
node slots on the free dim.  Per-level state H/C are mono-tiles
[128, 4*slots] (m-major columns) so each gate stage is ONE wide instruction
with nested access patterns instead of 4-12 small ones.  Leaf gates are
computed by the scalar engine directly from PSUM (bias bx+bs fused into the
activation), so leaf X projections are never materialized in SBUF.
"""

import os
import sys

sys.path.insert(0, "/opt/trn_rl_repo")

import numpy as np

import concourse.bass as bass
import concourse.bacc as bacc
import concourse.mybir as mybir
import concourse.tile as tile
from concourse.bass_utils import run_bass_kernel_spmd

F32 = mybir.dt.float32
F16 = mybir.dt.float16  # GEMM operand dtype (single-pass PE, 10-bit mantissa)
AF = mybir.ActivationFunctionType
ALU = mybir.AluOpType
AX = mybir.AxisListType

N = 4096
MEM = 512
IN_DIM = 512
NCORES = 8
P = 128
KT = 4  # contraction tiles (512 / 128)

# per-core column layout: internal+top region then compacted leaf region
OFF2, OFF1, OFF0 = 0, 128, 160
OFFT2, OFFT1, OFFT0 = 168, 184, 188
NI = 192                 # internal + top cols (3 pad at 189..191)
NLF = 384                # computed leaf cols (6 subtrees x 64)
NX = NI + NLF            # xin cols
NL3, NL2, NL1, NL0 = 512, 128, 32, 8

FOLD_FX = os.environ.get("KFOLD_FX", "1") == "1"
FOLD_IOU = os.environ.get("KFOLD_IOU", "1") == "1"
FP8_A = os.environ.get("KFP8", "0") == "1"  # phase-A inputs in fp8e4

LAST_RESULT = None  # BassKernelResults of the most recent run (for test.py)


def _core_heaps(c):
    heaps = np.full(NX, -1, dtype=np.int64)
    for s in range(8):
        t = 21 + 8 * s + c
        for a in range(16):
            heaps[OFF2 + 16 * s + a] = 16 * t + 5 + a
        for b in range(4):
            heaps[OFF1 + 4 * s + b] = 4 * t + 1 + b
        heaps[OFF0 + s] = t
    heaps[OFFT2:OFFT2 + 16] = np.arange(5, 21)
    heaps[OFFT1:OFFT1 + 4] = np.arange(1, 5)
    heaps[OFFT0] = 0
    for s in range(6):
        t = 21 + 8 * s + c
        for e in range(64):
            h = 64 * t + 21 + e
            heaps[NI + 64 * s + e] = h if h < N else -1
    return heaps


def _bcast4(ap, n):
    """broadcast the innermost dim 4x: [P, n] -> [P, n, 4(stride 0)]"""
    return bass.AP(tensor=ap.tensor, offset=ap.offset,
                   ap=list(ap.ap) + [[0, 4]])


def _build_program():
    nc = bacc.Bacc("TRN2", target_bir_lowering=False, debug=False)

    # packed f16 inputs: per-row [xin | wx] and [ws | wf] so each k-chunk
    # loads with one DMA (DMA issue on the sync queue is ~0.6us each)
    XDT = mybir.dt.float8e4 if FP8_A else F16
    xwx_d = nc.dram_tensor("xwx", [IN_DIM, NX + 4 * MEM], XDT,
                           kind="ExternalInput")
    swf_d = nc.dram_tensor("swf", [MEM, 4 * MEM], F16, kind="ExternalInput")
    # packed f32 per-partition inputs: [bxc | bxs | bf | cmask | ident]
    bp_d = nc.dram_tensor("bp", [P, 16 + 12 + 4 + NLF + P], F32,
                          kind="ExternalInput")
    sync_d = nc.dram_tensor("syncbuf", [1, 1], F32)
    syncg_d = nc.dram_tensor("syncg", [NCORES, 1], F32, addr_space="Shared")
    out_d = nc.dram_tensor("out", [P, KT], F32, kind="ExternalOutput")
    contrib_d = nc.dram_tensor("contrib", [P, 128], F16)
    gath_d = nc.dram_tensor("gath", [NCORES * P, 128], F16,
                            addr_space="Shared")

    with tile.TileContext(nc) as tc:
        with (
            tc.tile_pool(name="wpool", bufs=1) as wpool,
            tc.tile_pool(name="state", bufs=1) as state,
            tc.tile_pool(name="tmp", bufs=1) as tmp,
            tc.tile_pool(name="ps", bufs=1, space="PSUM") as ps,
        ):
            # ---- load everything (wx+xin first: phase A starts on them) ----
            xwx_s = [wpool.tile([P, NX + 4 * MEM], XDT, name="t", tag=f"xwx{k}")
                     for k in range(KT)]
            swf_s = [wpool.tile([P, 4 * MEM], F16, name="t", tag=f"swf{k}")
                     for k in range(KT)]
            in_s = [t[:, 0:NX] for t in xwx_s]
            wx_s = [t[:, NX:] for t in xwx_s]
            ws_s = [t[:, 0:3 * MEM] for t in swf_s]
            wf_s = [t[:, 3 * MEM:] for t in swf_s]
            bp_s = wpool.tile([P, 16 + 12 + 4 + NLF + P], F32, name="t", tag="bp")
            bxc_s = bp_s[:, 0:16]
            bxs_s = bp_s[:, 16:28]
            bf_s = bp_s[:, 28:32]
            cm_s = bp_s[:, 32:32 + NLF]
            idf_s = bp_s[:, 32 + NLF:]
            nc.sync.dma_start(bp_s[:], bp_d[:])
            id_s = wpool.tile([P, P], F16, name="t", tag="ident")
            nc.vector.tensor_copy(id_s[:], idf_s)
            for k in range(KT):
                r = slice(k * P, (k + 1) * P)
                nc.sync.dma_start(xwx_s[k][:], xwx_d[r, :])
            for k in range(KT):
                r = slice(k * P, (k + 1) * P)
                nc.sync.dma_start(swf_s[k][:], swf_d[r, :])
            # align the cores while the input DMAs stream: a 4-byte AllGather
            # absorbs the launch skew here, where the wait overlaps DMA time,
            # instead of at the root-gather where it is on the critical path.
            if os.environ.get("KSYNC", "1") == "1":
                with tc.high_priority():
                    nc.gpsimd.collective_compute(
                        "AllGather", ALU.bypass,
                        replica_groups=[list(range(NCORES))],
                        ins=[sync_d[:]],
                        outs=[syncg_d[:]],
                    )

            # ---- phase A (leaf): gates straight from PSUM, bias fused ----
            # 12 GEMMs [128, NLF]; sigma/tanh reads psum, writes mono gate tile
            IG = tmp.tile([P, KT * NLF], F32, name="t", tag="IG", bufs=1)
            OG = tmp.tile([P, KT * NLF], F32, name="t", tag="OG", bufs=1)
            UG = tmp.tile([P, KT * NLF], F32, name="t", tag="UG", bufs=1)
            leaf_jobs = []
            for m in range(KT):
                leaf_jobs += [(m, IG, AF.Sigmoid, m), (12 + m, UG, AF.Tanh, 8 + m),
                              (8 + m, OG, AF.Sigmoid, 4 + m)]
            for mc, gt, fn, bcol in leaf_jobs:
                m = mc % 4
                p_l = ps.tile([P, MEM], F32, name="t", tag="psA", bufs=4)
                for k in range(KT):
                    nc.tensor.matmul(
                        p_l[:, :NLF], wx_s[k][:, mc * P:(mc + 1) * P],
                        in_s[k][:, NI:NX],
                        start=(k == 0), stop=(k == KT - 1),
                    )
                nc.scalar.activation(gt[:, m * NLF:(m + 1) * NLF],
                                     p_l[:, :NLF],
                                     fn, bias=bxs_s[:, bcol:bcol + 1])

            # ---- phase A (internal): Xt mono [128, 16*NI], bias bxc fused.
            # f-block columns are emitted now; the i/o/u blocks are emitted
            # inside L2's f-path (mid_hook) so L2's f GEMMs start sooner.
            Xt = tmp.tile([P, 16 * NI], F16, name="t", tag="Xt", bufs=1)

            def emit_internal(mcs):
                for i, mc in enumerate(mcs):
                    p_i = ps.tile([P, MEM], F32, name="t", tag="psA", bufs=4)
                    for k in range(KT):
                        nc.tensor.matmul(
                            p_i[:, :NI], wx_s[k][:, mc * P:(mc + 1) * P],
                            in_s[k][:, 0:NI],
                            start=(k == 0), stop=(k == KT - 1),
                        )
                    if i % 2 == 0:
                        nc.vector.tensor_scalar_add(
                            Xt[:, mc * NI:(mc + 1) * NI],
                            p_i[:, :NI], bxc_s[:, mc:mc + 1])
                    else:
                        nc.scalar.activation(Xt[:, mc * NI:(mc + 1) * NI],
                                             p_i[:, :NI], AF.Identity,
                                             bias=bxc_s[:, mc:mc + 1])

            emit_internal([4, 5, 6, 7])

            # ---- leaf c/h into mono state H3/C3 [128, 4*512] ----
            H3 = state.tile([P, KT * NL3], F16, name="t", tag="H3")
            C3 = state.tile([P, KT * NL3], F32, name="t", tag="C3")
            # pad slots 384..511 of each m-chunk are zero
            padap = lambda t: bass.AP(tensor=t.tensor, offset=t.offset + NLF,
                                      ap=[t.ap[0], [NL3, KT], [1, NL3 - NLF]])
            nc.gpsimd.memset(padap(H3[:]), 0.0)
            nc.gpsimd.memset(padap(C3[:]), 0.0)
            CR = tmp.tile([P, KT * NLF], F32, name="t", tag="CR", bufs=1)
            nc.vector.tensor_mul(CR[:], IG[:], UG[:])
            # C3[:, m*512 + 0:384] = CR * cmask (mask broadcast over m)
            c3l = lambda t: bass.AP(tensor=t.tensor, offset=t.offset,
                                    ap=[t.ap[0], [NL3, KT], [1, NLF]])
            cmb = bass.AP(tensor=cm_s.tensor, offset=cm_s.offset,
                          ap=[cm_s.ap[0], [0, KT], [1, NLF]])
            crv = CR[:].rearrange("p (m e) -> p m e", m=KT)
            nc.gpsimd.tensor_mul(c3l(C3[:]), crv, cmb)
            THL = tmp.tile([P, KT * NLF], F32, name="t", tag="THL", bufs=1)
            nc.scalar.activation(THL[:].rearrange("p (m e) -> p m e", m=KT),
                                 c3l(C3[:]), AF.Tanh)
            nc.vector.tensor_mul(c3l(H3[:]),
                                 OG[:].rearrange("p (m e) -> p m e", m=KT),
                                 THL[:].rearrange("p (m e) -> p m e", m=KT))

            def level_step(n_par, x_off, Hc, Cc, hname, h_dtype=F16,
                           fh=None, mid_hook=None):
                """One fused ChildSumTreeLSTM level in T layout.
                Hc/Cc: mono child tiles [128, 4*nch]; returns mono Hp/Cp.
                fh: optional precomputed Wf.T @ Hc mono [128, 4*nch] (sbuf)."""
                nch = 4 * n_par
                # child-h sum first: it only needs Hc, and the vector queue is
                # in-order — emitting it before the f path lets the iou GEMMs
                # start as soon as the f GEMMs drain.
                CHS = tmp.tile([P, KT * n_par], F16, name="t", tag="CH")
                with nc.allow_low_precision("4-term child-h sum in f16"):
                    nc.vector.tensor_reduce(
                        CHS[:].rearrange("p (k n) -> p k n", k=KT),
                        Hc[:].rearrange("p (k n g) -> p k n g", k=KT, g=4),
                        axis=AX.X, op=ALU.add,
                    )
                # f = sigmoid(Wf.T @ Hc + fx + bf); fx folded into the psum
                # via an identity-stationary matmul; FCCS = sum4(f * Cc)
                F = tmp.tile([P, KT * nch], F16, name="t", tag="F")
                if fh is not None:
                    fxa = bass.AP(tensor=Xt.tensor,
                                  offset=Xt.offset + 4 * NI + x_off,
                                  ap=[Xt.ap[0], [NI, KT], [1, n_par], [0, 4]])
                    tf64 = tmp.tile([P, KT * nch], F32, name="t", tag="tf64",
                                    bufs=1)
                    nc.vector.tensor_add(
                        tf64[:].rearrange("p (m n g) -> p m n g", m=KT, g=4),
                        fh[:].rearrange("p (m n g) -> p m n g", m=KT, g=4),
                        fxa)
                    for m in range(KT):
                        nc.scalar.activation(F[:, m * nch:(m + 1) * nch],
                                             tf64[:, m * nch:(m + 1) * nch],
                                             AF.Sigmoid, bias=bf_s[:, m:m + 1])
                for m in range(KT if fh is None else 0):
                    p_f = ps.tile([P, MEM], F32, name="t", tag="psA", bufs=4)
                    for k in range(KT):
                        nc.tensor.matmul(
                            p_f[:, :nch], wf_s[k][:, m * P:(m + 1) * P],
                            Hc[:, k * nch:(k + 1) * nch],
                            start=(k == 0), stop=False, skip_group_check=True,
                        )
                    fx = bass.AP(tensor=Xt.tensor,
                                 offset=Xt.offset + (4 + m) * NI + x_off,
                                 ap=[Xt.ap[0], [1, n_par], [0, 4]])
                    nc.tensor.matmul(p_f[:, :nch], id_s[:], fx,
                                     start=False, stop=True,
                                     skip_group_check=True)
                    nc.scalar.activation(F[:, m * nch:(m + 1) * nch],
                                         p_f[:, :nch],
                                         AF.Sigmoid, bias=bf_s[:, m:m + 1])
                if mid_hook is not None:
                    mid_hook()
                # f*cc and its group-of-4 sum, pipelined per m-chunk so the
                # gpsimd mul and vector reduce overlap the next sigmoid
                FCC = tmp.tile([P, KT * nch], F16, name="t", tag="FCC")
                FCCS = tmp.tile([P, KT * n_par], F32, name="t", tag="FS")
                for m in range(KT):
                    cs = slice(m * nch, (m + 1) * nch)
                    nc.gpsimd.tensor_mul(FCC[:, cs], F[:, cs], Cc[:, cs])
                    nc.vector.tensor_reduce(
                        FCCS[:, m * n_par:(m + 1) * n_par].rearrange(
                            "p (o n) -> p o n", o=1),
                        FCC[:, cs].rearrange("p (o n g) -> p o n g", o=1, g=4),
                        axis=AX.X, op=ALU.add,
                    )
                # iou = Ws.T @ chs into one mono psum [128, 12*n_par], block
                # order i, u, o with activations fired per finished block: IU
                # and c need only i,u; the o gate is consumed last (for h).
                p_b = ps.tile([P, 12 * P], F32, name="t", tag="psB", bufs=1)
                GG = tmp.tile([P, 12 * n_par], F16, name="t", tag="GG")
                for mc in [0, 1, 2, 3, 8, 9, 10, 11, 4, 5, 6, 7]:
                    xt_mc = mc if mc < 4 else mc + 4
                    for k in range(KT):
                        nc.tensor.matmul(
                            p_b[:, mc * n_par:(mc + 1) * n_par],
                            ws_s[k][:, mc * P:(mc + 1) * P],
                            CHS[:, k * n_par:(k + 1) * n_par],
                            start=(k == 0), stop=False, skip_group_check=True,
                        )
                    xv = bass.AP(tensor=Xt.tensor,
                                 offset=Xt.offset + xt_mc * NI + x_off,
                                 ap=[Xt.ap[0], [1, n_par]])
                    nc.tensor.matmul(
                        p_b[:, mc * n_par:(mc + 1) * n_par],
                        id_s[:], xv, start=False, stop=True,
                        skip_group_check=True)
                    if mc == 3:
                        nc.scalar.activation(GG[:, :4 * n_par],
                                             p_b[:, :4 * n_par], AF.Sigmoid)
                    elif mc == 11:
                        nc.scalar.activation(GG[:, 8 * n_par:12 * n_par],
                                             p_b[:, 8 * n_par:12 * n_par],
                                             AF.Tanh)
                    elif mc == 7:
                        nc.scalar.activation(GG[:, 4 * n_par:8 * n_par],
                                             p_b[:, 4 * n_par:8 * n_par],
                                             AF.Sigmoid)
                IU = tmp.tile([P, KT * n_par], F32, name="t", tag="IU")
                nc.gpsimd.tensor_mul(IU[:], GG[:, :4 * n_par],
                                     GG[:, 8 * n_par:12 * n_par])
                Cp = state.tile([P, KT * n_par], F32, name="t", tag=f"C{hname}")
                nc.gpsimd.tensor_add(Cp[:], IU[:], FCCS[:])
                TH = tmp.tile([P, KT * n_par], F32, name="t", tag="TH")
                nc.scalar.activation(TH[:], Cp[:], AF.Tanh)
                Hp = state.tile([P, KT * n_par], h_dtype, name="t", tag=f"H{hname}")
                nc.gpsimd.tensor_mul(Hp[:], GG[:, 4 * n_par:8 * n_par], TH[:])
                return Hp, Cp

            H2, C2 = level_step(
                NL2, OFF2, H3, C3, "L2",
                mid_hook=lambda: emit_internal([0, 1, 2, 3] + list(range(8, 16))))
            H1, C1 = level_step(NL1, OFF1, H2, C2, "L1")
            H0, C0 = level_step(NL0, OFF0, H1, C1, "L0")

            # ---- gather the 64 subtree roots (h, c, Wf.T@h) to every core
            # contrib (f16) row p: [h f16 x32 | fh f16 x32 | c f32-as-2xf16].
            nc.sync.dma_start(contrib_d[:, 0:32], H0[:])
            nc.sync.dma_start(contrib_d[:, 64:128], C0[:].bitcast(F16))
            FH0 = tmp.tile([P, KT * NL0], F16, name="t", tag="FH0", bufs=1)
            for m in range(KT):
                p_h = ps.tile([P, MEM], F32, name="t", tag="psA", bufs=4)
                for k in range(KT):
                    nc.tensor.matmul(
                        p_h[:, :NL0], wf_s[k][:, m * P:(m + 1) * P],
                        H0[:, k * NL0:(k + 1) * NL0],
                        start=(k == 0), stop=(k == KT - 1),
                        skip_group_check=True,
                    )
                nc.vector.tensor_copy(FH0[:, m * NL0:(m + 1) * NL0],
                                      p_h[:, :NL0])
            nc.sync.dma_start(contrib_d[:, 32:64], FH0[:])
            nc.gpsimd.collective_compute(
                "AllGather", ALU.bypass,
                replica_groups=[list(range(NCORES))],
                ins=[contrib_d[:]],
                outs=[gath_d[:]],
            )
            # one contiguous load (256B runs), then engine-permute columns to
            # subtree order k = 8*s + c (T2 child column k).
            GR = state.tile([P, 2 * KT * 128], F16, name="t", tag="GR")
            H64 = state.tile([P, KT * 64], F16, name="t", tag="H64")
            C64 = state.tile([P, KT * 64], F32, name="t", tag="C64")
            FH64 = state.tile([P, KT * 64], F32, name="t", tag="FH64")
            nc.sync.dma_start(
                GR[:].rearrange("p (c j) -> p c j", c=NCORES),
                gath_d[:].rearrange("(c p) j -> p c j", c=NCORES))
            grf = GR[:].bitcast(F32)  # [128, 512]: c block at f32 col 32+
            perm_in = lambda base, off, cs: bass.AP(
                tensor=base.tensor, offset=base.offset + off,
                ap=[base.ap[0], [NL0, KT], [1, NL0], [cs, NCORES]])
            perm_out = lambda t: bass.AP(
                tensor=t.tensor, offset=t.offset,
                ap=[t.ap[0], [64, KT], [NL0, NL0], [1, NCORES]])
            nc.gpsimd.tensor_copy(perm_out(FH64[:]), perm_in(GR[:], 32, 128))
            nc.vector.tensor_copy(perm_out(H64[:]), perm_in(GR[:], 0, 128))
            nc.gpsimd.tensor_copy(perm_out(C64[:]), perm_in(grf, 32, 64))

            HT2, CT2 = level_step(16, OFFT2, H64, C64, "T2", fh=FH64)
            HT1, CT1 = level_step(4, OFFT1, HT2, CT2, "T1")
            HT0, _ = level_step(1, OFFT0, HT1, CT1, "T0", h_dtype=F32)
            nc.sync.dma_start(out_d[:], HT0[:])

    nc.compile()
    return nc


_NC_CACHE = None


def kernel(inputs, Wx, bx, Ws, bs, Wf, bf, children):
    global LAST_RESULT, _NC_CACHE
    inputs = np.asarray(inputs, np.float32)
    Wx = np.asarray(Wx, np.float32)
    bx = np.asarray(bx, np.float32)
    Ws = np.asarray(Ws, np.float32)
    bs = np.asarray(bs, np.float32)
    Wf = np.asarray(Wf, np.float32)
    bf = np.asarray(bf, np.float32)

    Wx_b = Wx.astype(np.float16)
    Ws_b = Ws.astype(np.float16)
    Wf_b = Wf.astype(np.float16)
    bxT = bx.reshape(16, P).T          # [128, 16] col mc
    bsT = bs.reshape(12, P).T          # [128, 12] col: i0..3 o0..3 u0..3
    bfT = np.ascontiguousarray(bf.reshape(4, P).T)
    # bxc: bx with bs folded into the i/o/u blocks (internal-X bias)
    bxc = bxT.copy()
    bxc[:, 0:4] += bsT[:, 0:4]
    bxc[:, 8:12] += bsT[:, 4:8]
    bxc[:, 12:16] += bsT[:, 8:12]
    bxc = np.ascontiguousarray(bxc)
    # bxs: leaf-gate bias (i,o,u blocks of bx + bs)
    bxs = np.concatenate(
        [bxT[:, 0:4] + bsT[:, 0:4], bxT[:, 8:12] + bsT[:, 4:8],
         bxT[:, 12:16] + bsT[:, 8:12]], axis=1)
    bxs = np.ascontiguousarray(bxs)

    swf = np.ascontiguousarray(np.concatenate([Ws_b, Wf_b], axis=1))
    in_maps = []
    for c in range(NCORES):
        heaps = _core_heaps(c)
        valid = heaps >= 0
        M = np.zeros((NX, IN_DIM), np.float32)
        M[valid] = inputs[N - 1 - heaps[valid]]
        xin = np.ascontiguousarray(M.T)
        mrow = valid[NI:].astype(np.float32)
        cmask = np.ascontiguousarray(np.tile(mrow[None, :], (P, 1)))
        xwx = np.concatenate([xin.astype(np.float16), Wx_b], axis=1)
        if FP8_A:
            import ml_dtypes
            xwx = xwx.astype(np.float32).astype(ml_dtypes.float8_e4m3)
        bp = np.concatenate([bxc, bxs, bfT, cmask,
                             np.eye(P, dtype=np.float32)], axis=1)
        in_maps.append({
            "xwx": np.ascontiguousarray(xwx), "swf": swf,
            "bp": np.ascontiguousarray(bp),
        })

    if _NC_CACHE is None:
        _NC_CACHE = _build_program()
    nc = _NC_CACHE

    res = run_bass_kernel_spmd(
        nc, in_maps, list(range(NCORES)),
        trace=bool(os.environ.get("BASS_TRACE")),
    )
    LAST_RESULT = res

    out = np.asarray(res.results[0]["out"])  # [128, 4]; h[m*128+p] = out[p, m]
    return np.ascontiguousarray(out.T.reshape(1, MEM))


# revision 26
# speedup vs baseline: 1.0404x; 1.0404x over previous
"""ChildSumTreeLSTM on 8 trn2 NeuronCores — v2 (fused).

Tree: reversed complete 4-ary heap (id = N-1-heap; heap j's children are
4j+1..4j+4).  The 64 depth-3 subtrees rooted at heap 21..84 are dealt to
cores stride-8 (subtree k -> core k%8, slot k//8) so that every core's REAL
level-6 leaves fit in its first 6 subtree slots (384 leaf columns); the
last 2 slots are always leafless, so the leaf level computes 384 columns
instead of 512.  Each core runs a uniform padded forest (levels 384-of-512
/128/32/8) and then every core redundantly computes the 21-node top tree
after a 32KB AllGather of the 64 subtree roots.

Layouts ("T layout"): mem (512 -> 4 partition chunks of 128) on partitions,
node slots on the free dim.  Per-level state H/C are mono-tiles
[128, 4*slots] (m-major columns) so each gate stage is ONE wide instruction
with nested access patterns instead of 4-12 small ones.  Leaf gates are
computed by the scalar engine directly from PSUM (bias bx+bs fused into the
activation), so leaf X projections are never materialized in SBUF.
"""

import os
import sys

sys.path.insert(0, "/opt/trn_rl_repo")

import numpy as np

import concourse.bass as bass
import concourse.bacc as bacc
import concourse.mybir as mybir
import concourse.tile as tile
from concourse.bass_utils import run_bass_kernel_spmd

F32 = mybir.dt.float32
F16 = mybir.dt.float16  # GEMM operand dtype (single-pass PE, 10-bit mantissa)
AF = mybir.ActivationFunctionType
ALU = mybir.AluOpType
AX = mybir.AxisListType

N = 4096
MEM = 512
IN_DIM = 512
NCORES = 8
P = 128
KT = 4  # contraction tiles (512 / 128)

# per-core column layout: internal+top region then compacted leaf region
OFF2, OFF1, OFF0 = 0, 128, 160
OFFT2, OFFT1, OFFT0 = 168, 184, 188
NI = 192                 # internal + top cols (3 pad at 189..191)
NLF = 384                # computed leaf cols (6 subtrees x 64)
NX = NI + NLF            # xin cols
NL3, NL2, NL1, NL0 = 512, 128, 32, 8

FOLD_FX = os.environ.get("KFOLD_FX", "1") == "1"
FOLD_IOU = os.environ.get("KFOLD_IOU", "1") == "1"
FP8_A = os.environ.get("KFP8", "0") == "1"  # phase-A inputs in fp8e4

LAST_RESULT = None  # BassKernelResults of the most recent run (for test.py)


def _core_heaps(c):
    heaps = np.full(NX, -1, dtype=np.int64)
    for s in range(8):
        t = 21 + 8 * s + c
        for a in range(16):
            heaps[OFF2 + 16 * s + a] = 16 * t + 5 + a
        for b in range(4):
            heaps[OFF1 + 4 * s + b] = 4 * t + 1 + b
        heaps[OFF0 + s] = t
    heaps[OFFT2:OFFT2 + 16] = np.arange(5, 21)
    heaps[OFFT1:OFFT1 + 4] = np.arange(1, 5)
    heaps[OFFT0] = 0
    for s in range(6):
        t = 21 + 8 * s + c
        for e in range(64):
            h = 64 * t + 21 + e
            heaps[NI + 64 * s + e] = h if h < N else -1
    return heaps


def _bcast4(ap, n):
    """broadcast the innermost dim 4x: [P, n] -> [P, n, 4(stride 0)]"""
    return bass.AP(tensor=ap.tensor, offset=ap.offset,
                   ap=list(ap.ap) + [[0, 4]])


def _build_program():
    nc = bacc.Bacc("TRN2", target_bir_lowering=False, debug=False)

    # packed f16 inputs: per-row [xin | wx] and [ws | wf] so each k-chunk
    # loads with one DMA (DMA issue on the sync queue is ~0.6us each)
    XDT = mybir.dt.float8e4 if FP8_A else F16
    xwx_d = nc.dram_tensor("xwx", [IN_DIM, NX + 4 * MEM], XDT,
                           kind="ExternalInput")
    swf_d = nc.dram_tensor("swf", [MEM, 4 * MEM], F16, kind="ExternalInput")
    # packed f32 per-partition inputs: [bxc | bxs | bf | cmask | ident]
    bp_d = nc.dram_tensor("bp", [P, 16 + 12 + 4 + NLF + P], F32,
                          kind="ExternalInput")
    sync_d = nc.dram_tensor("syncbuf", [1, 1], F32)
    syncg_d = nc.dram_tensor("syncg", [NCORES, 1], F32, addr_space="Shared")
    out_d = nc.dram_tensor("out", [P, KT], F32, kind="ExternalOutput")
    contrib_d = nc.dram_tensor("contrib", [P, 128], F16)
    gath_d = nc.dram_tensor("gath", [NCORES * P, 128], F16,
                            addr_space="Shared")

    with tile.TileContext(nc) as tc:
        with (
            tc.tile_pool(name="wpool", bufs=1) as wpool,
            tc.tile_pool(name="state", bufs=1) as state,
            tc.tile_pool(name="tmp", bufs=1) as tmp,
            tc.tile_pool(name="ps", bufs=1, space="PSUM") as ps,
        ):
            # ---- load everything (wx+xin first: phase A starts on them) ----
            xwx_s = [wpool.tile([P, NX + 4 * MEM], XDT, name="t", tag=f"xwx{k}")
                     for k in range(KT)]
            swf_s = [wpool.tile([P, 4 * MEM], F16, name="t", tag=f"swf{k}")
                     for k in range(KT)]
            in_s = [t[:, 0:NX] for t in xwx_s]
            wx_s = [t[:, NX:] for t in xwx_s]
            ws_s = [t[:, 0:3 * MEM] for t in swf_s]
            wf_s = [t[:, 3 * MEM:] for t in swf_s]
            bp_s = wpool.tile([P, 16 + 12 + 4 + NLF + P], F32, name="t", tag="bp")
            bxc_s = bp_s[:, 0:16]
            bxs_s = bp_s[:, 16:28]
            bf_s = bp_s[:, 28:32]
            cm_s = bp_s[:, 32:32 + NLF]
            idf_s = bp_s[:, 32 + NLF:]
            nc.sync.dma_start(bp_s[:], bp_d[:])
            id_s = wpool.tile([P, P], F16, name="t", tag="ident")
            nc.vector.tensor_copy(id_s[:], idf_s)
            for k in range(KT):
                r = slice(k * P, (k + 1) * P)
                nc.sync.dma_start(xwx_s[k][:], xwx_d[r, :])
            for k in range(KT):
                r = slice(k * P, (k + 1) * P)
                nc.sync.dma_start(swf_s[k][:], swf_d[r, :])
            # align the cores while the input DMAs stream: a 4-byte AllGather
            # absorbs the launch skew here, where the wait overlaps DMA time,
            # instead of at the root-gather where it is on the critical path.
            if os.environ.get("KSYNC", "1") == "1":
                with tc.high_priority():
                    nc.gpsimd.collective_compute(
                        "AllGather", ALU.bypass,
                        replica_groups=[list(range(NCORES))],
                        ins=[sync_d[:]],
                        outs=[syncg_d[:]],
                    )

            # ---- phase A (leaf): gates straight from PSUM, bias fused ----
            # 12 GEMMs [128, NLF]; sigma/tanh reads psum, writes mono gate tile
            IG = tmp.tile([P, KT * NLF], F32, name="t", tag="IG", bufs=1)
            OG = tmp.tile([P, KT * NLF], F32, name="t", tag="OG", bufs=1)
            UG = tmp.tile([P, KT * NLF], F32, name="t", tag="UG", bufs=1)
            leaf_jobs = []
            for m in range(KT):
                leaf_jobs += [(m, IG, AF.Sigmoid, m), (12 + m, UG, AF.Tanh, 8 + m),
                              (8 + m, OG, AF.Sigmoid, 4 + m)]
            for mc, gt, fn, bcol in leaf_jobs:
                m = mc % 4
                p_l = ps.tile([P, MEM], F32, name="t", tag="psA", bufs=4)
                for k in range(KT):
                    nc.tensor.matmul(
                        p_l[:, :NLF], wx_s[k][:, mc * P:(mc + 1) * P],
                        in_s[k][:, NI:NX],
                        start=(k == 0), stop=(k == KT - 1),
                    )
                nc.scalar.activation(gt[:, m * NLF:(m + 1) * NLF],
                                     p_l[:, :NLF],
                                     fn, bias=bxs_s[:, bcol:bcol + 1])

            # ---- phase A (internal): Xt mono [128, 16*NI], bias bxc fused.
            # f-block columns are emitted now; the i/o/u blocks are emitted
            # inside L2's f-path (mid_hook) so L2's f GEMMs start sooner.
            Xt = tmp.tile([P, 16 * NI], F16, name="t", tag="Xt", bufs=1)

            def emit_internal(mcs):
                for i, mc in enumerate(mcs):
                    p_i = ps.tile([P, MEM], F32, name="t", tag="psA", bufs=4)
                    for k in range(KT):
                        nc.tensor.matmul(
                            p_i[:, :NI], wx_s[k][:, mc * P:(mc + 1) * P],
                            in_s[k][:, 0:NI],
                            start=(k == 0), stop=(k == KT - 1),
                        )
                    if i % 2 == 0:
                        nc.vector.tensor_scalar_add(
                            Xt[:, mc * NI:(mc + 1) * NI],
                            p_i[:, :NI], bxc_s[:, mc:mc + 1])
                    else:
                        nc.scalar.activation(Xt[:, mc * NI:(mc + 1) * NI],
                                             p_i[:, :NI], AF.Identity,
                                             bias=bxc_s[:, mc:mc + 1])

            emit_internal([4, 5, 6, 7] + [0, 1, 2, 3] + list(range(8, 16)))

            # ---- leaf c/h into mono state H3/C3 [128, 4*512] ----
            H3 = state.tile([P, KT * NL3], F16, name="t", tag="H3")
            C3 = state.tile([P, KT * NL3], F32, name="t", tag="C3")
            # pad slots 384..511 of each m-chunk are zero
            padap = lambda t: bass.AP(tensor=t.tensor, offset=t.offset + NLF,
                                      ap=[t.ap[0], [NL3, KT], [1, NL3 - NLF]])
            nc.gpsimd.memset(padap(H3[:]), 0.0)
            nc.gpsimd.memset(padap(C3[:]), 0.0)
            CR = tmp.tile([P, KT * NLF], F32, name="t", tag="CR", bufs=1)
            nc.vector.tensor_mul(CR[:], IG[:], UG[:])
            # C3[:, m*512 + 0:384] = CR * cmask (mask broadcast over m)
            c3l = lambda t: bass.AP(tensor=t.tensor, offset=t.offset,
                                    ap=[t.ap[0], [NL3, KT], [1, NLF]])
            cmb = bass.AP(tensor=cm_s.tensor, offset=cm_s.offset,
                          ap=[cm_s.ap[0], [0, KT], [1, NLF]])
            crv = CR[:].rearrange("p (m e) -> p m e", m=KT)
            nc.gpsimd.tensor_mul(c3l(C3[:]), crv, cmb)
            THL = tmp.tile([P, KT * NLF], F32, name="t", tag="THL", bufs=1)
            nc.scalar.activation(THL[:].rearrange("p (m e) -> p m e", m=KT),
                                 c3l(C3[:]), AF.Tanh)
            nc.vector.tensor_mul(c3l(H3[:]),
                                 OG[:].rearrange("p (m e) -> p m e", m=KT),
                                 THL[:].rearrange("p (m e) -> p m e", m=KT))

            def level_step(n_par, x_off, Hc, Cc, hname, h_dtype=F16,
                           fh=None, mid_hook=None):
                """One fused ChildSumTreeLSTM level in T layout.
                Hc/Cc: mono child tiles [128, 4*nch]; returns mono Hp/Cp.
                fh: optional precomputed Wf.T @ Hc mono [128, 4*nch] (sbuf)."""
                nch = 4 * n_par
                # child-h sum first: it only needs Hc, and the vector queue is
                # in-order — emitting it before the f path lets the iou GEMMs
                # start as soon as the f GEMMs drain.
                CHS = tmp.tile([P, KT * n_par], F16, name="t", tag="CH")
                with nc.allow_low_precision("4-term child-h sum in f16"):
                    nc.vector.tensor_reduce(
                        CHS[:].rearrange("p (k n) -> p k n", k=KT),
                        Hc[:].rearrange("p (k n g) -> p k n g", k=KT, g=4),
                        axis=AX.X, op=ALU.add,
                    )
                # f = sigmoid(Wf.T @ Hc + fx + bf); fx folded into the psum
                # via an identity-stationary matmul; FCCS = sum4(f * Cc)
                F = tmp.tile([P, KT * nch], F16, name="t", tag="F")
                if fh is not None:
                    fxa = bass.AP(tensor=Xt.tensor,
                                  offset=Xt.offset + 4 * NI + x_off,
                                  ap=[Xt.ap[0], [NI, KT], [1, n_par], [0, 4]])
                    tf64 = tmp.tile([P, KT * nch], F32, name="t", tag="tf64",
                                    bufs=1)
                    nc.vector.tensor_add(
                        tf64[:].rearrange("p (m n g) -> p m n g", m=KT, g=4),
                        fh[:].rearrange("p (m n g) -> p m n g", m=KT, g=4),
                        fxa)
                    for m in range(KT):
                        nc.scalar.activation(F[:, m * nch:(m + 1) * nch],
                                             tf64[:, m * nch:(m + 1) * nch],
                                             AF.Sigmoid, bias=bf_s[:, m:m + 1])
                for m in range(KT if fh is None else 0):
                    p_f = ps.tile([P, MEM], F32, name="t", tag="psA", bufs=4)
                    for k in range(KT):
                        nc.tensor.matmul(
                            p_f[:, :nch], wf_s[k][:, m * P:(m + 1) * P],
                            Hc[:, k * nch:(k + 1) * nch],
                            start=(k == 0), stop=False, skip_group_check=True,
                        )
                    fx = bass.AP(tensor=Xt.tensor,
                                 offset=Xt.offset + (4 + m) * NI + x_off,
                                 ap=[Xt.ap[0], [1, n_par], [0, 4]])
                    nc.tensor.matmul(p_f[:, :nch], id_s[:], fx,
                                     start=False, stop=True,
                                     skip_group_check=True)
                    nc.scalar.activation(F[:, m * nch:(m + 1) * nch],
                                         p_f[:, :nch],
                                         AF.Sigmoid, bias=bf_s[:, m:m + 1])
                if mid_hook is not None:
                    mid_hook()
                # f*cc and its group-of-4 sum, pipelined per m-chunk so the
                # gpsimd mul and vector reduce overlap the next sigmoid
                FCC = tmp.tile([P, KT * nch], F16, name="t", tag="FCC")
                FCCS = tmp.tile([P, KT * n_par], F32, name="t", tag="FS")
                for m in range(KT):
                    cs = slice(m * nch, (m + 1) * nch)
                    nc.gpsimd.tensor_mul(FCC[:, cs], F[:, cs], Cc[:, cs])
                    nc.vector.tensor_reduce(
                        FCCS[:, m * n_par:(m + 1) * n_par].rearrange(
                            "p (o n) -> p o n", o=1),
                        FCC[:, cs].rearrange("p (o n g) -> p o n g", o=1, g=4),
                        axis=AX.X, op=ALU.add,
                    )
                # iou = Ws.T @ chs into one mono psum [128, 12*n_par], block
                # order i, u, o with activations fired per finished block: IU
                # and c need only i,u; the o gate is consumed last (for h).
                p_b = ps.tile([P, 12 * P], F32, name="t", tag="psB", bufs=1)
                GG = tmp.tile([P, 12 * n_par], F16, name="t", tag="GG")
                for mc in [0, 1, 2, 3, 8, 9, 10, 11, 4, 5, 6, 7]:
                    xt_mc = mc if mc < 4 else mc + 4
                    for k in range(KT):
                        nc.tensor.matmul(
                            p_b[:, mc * n_par:(mc + 1) * n_par],
                            ws_s[k][:, mc * P:(mc + 1) * P],
                            CHS[:, k * n_par:(k + 1) * n_par],
                            start=(k == 0), stop=False, skip_group_check=True,
                        )
                    xv = bass.AP(tensor=Xt.tensor,
                                 offset=Xt.offset + xt_mc * NI + x_off,
                                 ap=[Xt.ap[0], [1, n_par]])
                    nc.tensor.matmul(
                        p_b[:, mc * n_par:(mc + 1) * n_par],
                        id_s[:], xv, start=False, stop=True,
                        skip_group_check=True)
                    if mc == 3:
                        nc.scalar.activation(GG[:, :4 * n_par],
                                             p_b[:, :4 * n_par], AF.Sigmoid)
                    elif mc == 11:
                        nc.scalar.activation(GG[:, 8 * n_par:12 * n_par],
                                             p_b[:, 8 * n_par:12 * n_par],
                                             AF.Tanh)
                    elif mc == 7:
                        nc.scalar.activation(GG[:, 4 * n_par:8 * n_par],
                                             p_b[:, 4 * n_par:8 * n_par],
                                             AF.Sigmoid)
                IU = tmp.tile([P, KT * n_par], F32, name="t", tag="IU")
                nc.gpsimd.tensor_mul(IU[:], GG[:, :4 * n_par],
                                     GG[:, 8 * n_par:12 * n_par])
                Cp = state.tile([P, KT * n_par], F32, name="t", tag=f"C{hname}")
                nc.gpsimd.tensor_add(Cp[:], IU[:], FCCS[:])
                TH = tmp.tile([P, KT * n_par], F32, name="t", tag="TH")
                nc.scalar.activation(TH[:], Cp[:], AF.Tanh)
                Hp = state.tile([P, KT * n_par], h_dtype, name="t", tag=f"H{hname}")
                nc.gpsimd.tensor_mul(Hp[:], GG[:, 4 * n_par:8 * n_par], TH[:])
                return Hp, Cp

            H2, C2 = level_step(NL2, OFF2, H3, C3, "L2")
            H1, C1 = level_step(NL1, OFF1, H2, C2, "L1")
            H0, C0 = level_step(NL0, OFF0, H1, C1, "L0")

            # ---- gather the 64 subtree roots (h, c, Wf.T@h) to every core
            # contrib (f16) row p: [h f16 x32 | fh f16 x32 | c f32-as-2xf16].
            nc.sync.dma_start(contrib_d[:, 0:32], H0[:])
            nc.sync.dma_start(contrib_d[:, 64:128], C0[:].bitcast(F16))
            FH0 = tmp.tile([P, KT * NL0], F16, name="t", tag="FH0", bufs=1)
            for m in range(KT):
                p_h = ps.tile([P, MEM], F32, name="t", tag="psA", bufs=4)
                for k in range(KT):
                    nc.tensor.matmul(
                        p_h[:, :NL0], wf_s[k][:, m * P:(m + 1) * P],
                        H0[:, k * NL0:(k + 1) * NL0],
                        start=(k == 0), stop=(k == KT - 1),
                        skip_group_check=True,
                    )
                nc.vector.tensor_copy(FH0[:, m * NL0:(m + 1) * NL0],
                                      p_h[:, :NL0])
            nc.sync.dma_start(contrib_d[:, 32:64], FH0[:])
            nc.gpsimd.collective_compute(
                "AllGather", ALU.bypass,
                replica_groups=[list(range(NCORES))],
                ins=[contrib_d[:]],
                outs=[gath_d[:]],
            )
            # one contiguous load (256B runs), then engine-permute columns to
            # subtree order k = 8*s + c (T2 child column k).
            GR = state.tile([P, 2 * KT * 128], F16, name="t", tag="GR")
            H64 = state.tile([P, KT * 64], F16, name="t", tag="H64")
            C64 = state.tile([P, KT * 64], F32, name="t", tag="C64")
            FH64 = state.tile([P, KT * 64], F32, name="t", tag="FH64")
            nc.sync.dma_start(
                GR[:].rearrange("p (c j) -> p c j", c=NCORES),
                gath_d[:].rearrange("(c p) j -> p c j", c=NCORES))
            grf = GR[:].bitcast(F32)  # [128, 512]: c block at f32 col 32+
            perm_in = lambda base, off, cs: bass.AP(
                tensor=base.tensor, offset=base.offset + off,
                ap=[base.ap[0], [NL0, KT], [1, NL0], [cs, NCORES]])
            perm_out = lambda t: bass.AP(
                tensor=t.tensor, offset=t.offset,
                ap=[t.ap[0], [64, KT], [NL0, NL0], [1, NCORES]])
            nc.gpsimd.tensor_copy(perm_out(FH64[:]), perm_in(GR[:], 32, 128))
            nc.vector.tensor_copy(perm_out(H64[:]), perm_in(GR[:], 0, 128))
            nc.gpsimd.tensor_copy(perm_out(C64[:]), perm_in(grf, 32, 64))

            HT2, CT2 = level_step(16, OFFT2, H64, C64, "T2", fh=FH64)
            HT1, CT1 = level_step(4, OFFT1, HT2, CT2, "T1")
            HT0, _ = level_step(1, OFFT0, HT1, CT1, "T0", h_dtype=F32)
            nc.sync.dma_start(out_d[:], HT0[:])

    nc.compile()
    return nc


_NC_CACHE = None


def kernel(inputs, Wx, bx, Ws, bs, Wf, bf, children):
    global LAST_RESULT, _NC_CACHE
    inputs = np.asarray(inputs, np.float32)
    Wx = np.asarray(Wx, np.float32)
    bx = np.asarray(bx, np.float32)
    Ws = np.asarray(Ws, np.float32)
    bs = np.asarray(bs, np.float32)
    Wf = np.asarray(Wf, np.float32)
    bf = np.asarray(bf, np.float32)

    Wx_b = Wx.astype(np.float16)
    Ws_b = Ws.astype(np.float16)
    Wf_b = Wf.astype(np.float16)
    bxT = bx.reshape(16, P).T          # [128, 16] col mc
    bsT = bs.reshape(12, P).T          # [128, 12] col: i0..3 o0..3 u0..3
    bfT = np.ascontiguousarray(bf.reshape(4, P).T)
    # bxc: bx with bs folded into the i/o/u blocks (internal-X bias)
    bxc = bxT.copy()
    bxc[:, 0:4] += bsT[:, 0:4]
    bxc[:, 8:12] += bsT[:, 4:8]
    bxc[:, 12:16] += bsT[:, 8:12]
    bxc = np.ascontiguousarray(bxc)
    # bxs: leaf-gate bias (i,o,u blocks of bx + bs)
    bxs = np.concatenate(
        [bxT[:, 0:4] + bsT[:, 0:4], bxT[:, 8:12] + bsT[:, 4:8],
         bxT[:, 12:16] + bsT[:, 8:12]], axis=1)
    bxs = np.ascontiguousarray(bxs)

    swf = np.ascontiguousarray(np.concatenate([Ws_b, Wf_b], axis=1))
    in_maps = []
    for c in range(NCORES):
        heaps = _core_heaps(c)
        valid = heaps >= 0
        M = np.zeros((NX, IN_DIM), np.float32)
        M[valid] = inputs[N - 1 - heaps[valid]]
        xin = np.ascontiguousarray(M.T)
        mrow = valid[NI:].astype(np.float32)
        cmask = np.ascontiguousarray(np.tile(mrow[None, :], (P, 1)))
        xwx = np.concatenate([xin.astype(np.float16), Wx_b], axis=1)
        if FP8_A:
            import ml_dtypes
            xwx = xwx.astype(np.float32).astype(ml_dtypes.float8_e4m3)
        bp = np.concatenate([bxc, bxs, bfT, cmask,
                             np.eye(P, dtype=np.float32)], axis=1)
        in_maps.append({
            "xwx": np.ascontiguousarray(xwx), "swf": swf,
            "bp": np.ascontiguousarray(bp),
        })

    if _NC_CACHE is None:
        _NC_CACHE = _build_program()
    nc = _NC_CACHE

    res = run_bass_kernel_spmd(
        nc, in_maps, list(range(NCORES)),
        trace=bool(os.environ.get("BASS_TRACE")),
    )
    LAST_RESULT = res

    out = np.asarray(res.results[0]["out"])  # [128, 4]; h[m*128+p] = out[p, m]
    return np.ascontiguousarray(out.T.reshape(1, MEM))


# revision 27
# speedup vs baseline: 1.0530x; 1.0122x over previous
"""ChildSumTreeLSTM on 8 trn2 NeuronCores — v2 (fused).

Tree: reversed complete 4-ary heap (id = N-1-heap; heap j's children are
4j+1..4j+4).  The 64 depth-3 subtrees rooted at heap 21..84 are dealt to
cores stride-8 (subtree k -> core k%8, slot k//8) so that every core's REAL
level-6 leaves fit in its first 6 subtree slots (384 leaf columns); the
last 2 slots are always leafless, so the leaf level computes 384 columns
instead of 512.  Each core runs a uniform padded forest (levels 384-of-512
/128/32/8) and then every core redundantly computes the 21-node top tree
after a 32KB AllGather of the 64 subtree roots.

Layouts ("T layout"): mem (512 -> 4 partition chunks of 128) on partitions,
node slots on the free dim.  Per-level state H/C are mono-tiles
[128, 4*slots] (m-major columns) so each gate stage is ONE wide instruction
with nested access patterns instead of 4-12 small ones.  Leaf gates are
computed by the scalar engine directly from PSUM (bias bx+bs fused into the
activation), so leaf X projections are never materialized in SBUF.
"""

import os
import sys

sys.path.insert(0, "/opt/trn_rl_repo")

import numpy as np

import concourse.bass as bass
import concourse.bacc as bacc
import concourse.mybir as mybir
import concourse.tile as tile
from concourse.bass_utils import run_bass_kernel_spmd

F32 = mybir.dt.float32
F16 = mybir.dt.float16  # GEMM operand dtype (single-pass PE, 10-bit mantissa)
AF = mybir.ActivationFunctionType
ALU = mybir.AluOpType
AX = mybir.AxisListType

N = 4096
MEM = 512
IN_DIM = 512
NCORES = 8
P = 128
KT = 4  # contraction tiles (512 / 128)

# per-core column layout: internal+top region then compacted leaf region
OFF2, OFF1, OFF0 = 0, 128, 160
OFFT2, OFFT1, OFFT0 = 168, 184, 188
NI = 192                 # internal + top cols (3 pad at 189..191)
NLF = 384                # computed leaf cols (6 subtrees x 64)
NX = NI + NLF            # xin cols
NL3, NL2, NL1, NL0 = 512, 128, 32, 8

FOLD_FX = os.environ.get("KFOLD_FX", "1") == "1"
FOLD_IOU = os.environ.get("KFOLD_IOU", "1") == "1"
FP8_A = os.environ.get("KFP8", "0") == "1"  # phase-A inputs in fp8e4

LAST_RESULT = None  # BassKernelResults of the most recent run (for test.py)


def _core_heaps(c):
    heaps = np.full(NX, -1, dtype=np.int64)
    for s in range(8):
        t = 21 + 8 * s + c
        for a in range(16):
            heaps[OFF2 + 16 * s + a] = 16 * t + 5 + a
        for b in range(4):
            heaps[OFF1 + 4 * s + b] = 4 * t + 1 + b
        heaps[OFF0 + s] = t
    heaps[OFFT2:OFFT2 + 16] = np.arange(5, 21)
    heaps[OFFT1:OFFT1 + 4] = np.arange(1, 5)
    heaps[OFFT0] = 0
    for s in range(6):
        t = 21 + 8 * s + c
        for e in range(64):
            h = 64 * t + 21 + e
            heaps[NI + 64 * s + e] = h if h < N else -1
    return heaps


def _bcast4(ap, n):
    """broadcast the innermost dim 4x: [P, n] -> [P, n, 4(stride 0)]"""
    return bass.AP(tensor=ap.tensor, offset=ap.offset,
                   ap=list(ap.ap) + [[0, 4]])


def _build_program():
    nc = bacc.Bacc("TRN2", target_bir_lowering=False, debug=False)

    # packed f16 inputs: per-row [xin | wx] and [ws | wf] so each k-chunk
    # loads with one DMA (DMA issue on the sync queue is ~0.6us each)
    XDT = mybir.dt.float8e4 if FP8_A else F16
    xwx_d = nc.dram_tensor("xwx", [IN_DIM, NX + 4 * MEM], XDT,
                           kind="ExternalInput")
    swf_d = nc.dram_tensor("swf", [MEM, 4 * MEM], F16, kind="ExternalInput")
    # packed f32 per-partition inputs: [bxc | bxs | bf | cmask | ident]
    bp_d = nc.dram_tensor("bp", [P, 16 + 12 + 4 + NLF + P], F32,
                          kind="ExternalInput")
    sync_d = nc.dram_tensor("syncbuf", [1, 1], F32)
    syncg_d = nc.dram_tensor("syncg", [NCORES, 1], F32, addr_space="Shared")
    out_d = nc.dram_tensor("out", [P, KT], F32, kind="ExternalOutput")
    contrib_d = nc.dram_tensor("contrib", [P, 128], F16)
    gath_d = nc.dram_tensor("gath", [NCORES * P, 128], F16,
                            addr_space="Shared")

    with tile.TileContext(nc) as tc:
        with (
            tc.tile_pool(name="wpool", bufs=1) as wpool,
            tc.tile_pool(name="state", bufs=1) as state,
            tc.tile_pool(name="tmp", bufs=1) as tmp,
            tc.tile_pool(name="ps", bufs=1, space="PSUM") as ps,
        ):
            # ---- load everything (wx+xin first: phase A starts on them) ----
            xwx_s = [wpool.tile([P, NX + 4 * MEM], XDT, name="t", tag=f"xwx{k}")
                     for k in range(KT)]
            swf_s = [wpool.tile([P, 4 * MEM], F16, name="t", tag=f"swf{k}")
                     for k in range(KT)]
            in_s = [t[:, 0:NX] for t in xwx_s]
            wx_s = [t[:, NX:] for t in xwx_s]
            ws_s = [t[:, 0:3 * MEM] for t in swf_s]
            wf_s = [t[:, 3 * MEM:] for t in swf_s]
            bp_s = wpool.tile([P, 16 + 12 + 4 + NLF + P], F32, name="t", tag="bp")
            bxc_s = bp_s[:, 0:16]
            bxs_s = bp_s[:, 16:28]
            bf_s = bp_s[:, 28:32]
            cm_s = bp_s[:, 32:32 + NLF]
            idf_s = bp_s[:, 32 + NLF:]
            for k in range(KT):
                r = slice(k * P, (k + 1) * P)
                nc.sync.dma_start(xwx_s[k][:], xwx_d[r, :])
            nc.sync.dma_start(bp_s[:], bp_d[:])
            id_s = wpool.tile([P, P], F16, name="t", tag="ident")
            nc.vector.tensor_copy(id_s[:], idf_s)
            for k in range(KT):
                r = slice(k * P, (k + 1) * P)
                nc.sync.dma_start(swf_s[k][:], swf_d[r, :])
            # align the cores while the input DMAs stream: a 4-byte AllGather
            # absorbs the launch skew here, where the wait overlaps DMA time,
            # instead of at the root-gather where it is on the critical path.
            if os.environ.get("KSYNC", "1") == "1":
                with tc.high_priority():
                    nc.gpsimd.collective_compute(
                        "AllGather", ALU.bypass,
                        replica_groups=[list(range(NCORES))],
                        ins=[sync_d[:]],
                        outs=[syncg_d[:]],
                    )

            # ---- phase A (leaf): gates straight from PSUM, bias fused ----
            # 12 GEMMs [128, NLF]; sigma/tanh reads psum, writes mono gate tile
            IG = tmp.tile([P, KT * NLF], F32, name="t", tag="IG", bufs=1)
            OG = tmp.tile([P, KT * NLF], F32, name="t", tag="OG", bufs=1)
            UG = tmp.tile([P, KT * NLF], F32, name="t", tag="UG", bufs=1)
            leaf_jobs = []
            for m in range(KT):
                leaf_jobs += [(m, IG, AF.Sigmoid, m), (12 + m, UG, AF.Tanh, 8 + m),
                              (8 + m, OG, AF.Sigmoid, 4 + m)]
            for mc, gt, fn, bcol in leaf_jobs:
                m = mc % 4
                p_l = ps.tile([P, MEM], F32, name="t", tag="psA", bufs=4)
                for k in range(KT):
                    nc.tensor.matmul(
                        p_l[:, :NLF], wx_s[k][:, mc * P:(mc + 1) * P],
                        in_s[k][:, NI:NX],
                        start=(k == 0), stop=(k == KT - 1),
                    )
                nc.scalar.activation(gt[:, m * NLF:(m + 1) * NLF],
                                     p_l[:, :NLF],
                                     fn, bias=bxs_s[:, bcol:bcol + 1])

            # ---- leaf c/h into mono state H3/C3 [128, 4*512] ----
            H3 = state.tile([P, KT * NL3], F16, name="t", tag="H3")
            C3 = state.tile([P, KT * NL3], F32, name="t", tag="C3")
            # pad slots 384..511 of each m-chunk are zero
            padap = lambda t: bass.AP(tensor=t.tensor, offset=t.offset + NLF,
                                      ap=[t.ap[0], [NL3, KT], [1, NL3 - NLF]])
            nc.gpsimd.memset(padap(H3[:]), 0.0)
            nc.gpsimd.memset(padap(C3[:]), 0.0)
            CR = tmp.tile([P, KT * NLF], F32, name="t", tag="CR", bufs=1)
            nc.vector.tensor_mul(CR[:], IG[:], UG[:])
            # C3[:, m*512 + 0:384] = CR * cmask (mask broadcast over m)
            c3l = lambda t: bass.AP(tensor=t.tensor, offset=t.offset,
                                    ap=[t.ap[0], [NL3, KT], [1, NLF]])
            cmb = bass.AP(tensor=cm_s.tensor, offset=cm_s.offset,
                          ap=[cm_s.ap[0], [0, KT], [1, NLF]])
            crv = CR[:].rearrange("p (m e) -> p m e", m=KT)
            nc.gpsimd.tensor_mul(c3l(C3[:]), crv, cmb)
            THL = tmp.tile([P, KT * NLF], F32, name="t", tag="THL", bufs=1)
            nc.scalar.activation(THL[:].rearrange("p (m e) -> p m e", m=KT),
                                 c3l(C3[:]), AF.Tanh)
            nc.vector.tensor_mul(c3l(H3[:]),
                                 OG[:].rearrange("p (m e) -> p m e", m=KT),
                                 THL[:].rearrange("p (m e) -> p m e", m=KT))

            # ---- phase A (internal): Xt mono [128, 16*NI], bias bxc fused.
            # f-block columns are emitted now; the i/o/u blocks are emitted
            # inside L2's f-path (mid_hook) so L2's f GEMMs start sooner.
            Xt = tmp.tile([P, 16 * NI], F16, name="t", tag="Xt", bufs=1)

            def emit_internal(mcs):
                for i, mc in enumerate(mcs):
                    p_i = ps.tile([P, MEM], F32, name="t", tag="psA", bufs=4)
                    for k in range(KT):
                        nc.tensor.matmul(
                            p_i[:, :NI], wx_s[k][:, mc * P:(mc + 1) * P],
                            in_s[k][:, 0:NI],
                            start=(k == 0), stop=(k == KT - 1),
                        )
                    if i % 2 == 0:
                        nc.vector.tensor_scalar_add(
                            Xt[:, mc * NI:(mc + 1) * NI],
                            p_i[:, :NI], bxc_s[:, mc:mc + 1])
                    else:
                        nc.scalar.activation(Xt[:, mc * NI:(mc + 1) * NI],
                                             p_i[:, :NI], AF.Identity,
                                             bias=bxc_s[:, mc:mc + 1])

            emit_internal([4, 5, 6, 7] + [0, 1, 2, 3] + list(range(8, 16)))

            def level_step(n_par, x_off, Hc, Cc, hname, h_dtype=F16,
                           fh=None, mid_hook=None):
                """One fused ChildSumTreeLSTM level in T layout.
                Hc/Cc: mono child tiles [128, 4*nch]; returns mono Hp/Cp.
                fh: optional precomputed Wf.T @ Hc mono [128, 4*nch] (sbuf)."""
                nch = 4 * n_par
                # child-h sum first: it only needs Hc, and the vector queue is
                # in-order — emitting it before the f path lets the iou GEMMs
                # start as soon as the f GEMMs drain.
                CHS = tmp.tile([P, KT * n_par], F16, name="t", tag="CH")
                with nc.allow_low_precision("4-term child-h sum in f16"):
                    nc.vector.tensor_reduce(
                        CHS[:].rearrange("p (k n) -> p k n", k=KT),
                        Hc[:].rearrange("p (k n g) -> p k n g", k=KT, g=4),
                        axis=AX.X, op=ALU.add,
                    )
                # f = sigmoid(Wf.T @ Hc + fx + bf); fx folded into the psum
                # via an identity-stationary matmul; FCCS = sum4(f * Cc)
                F = tmp.tile([P, KT * nch], F16, name="t", tag="F")
                if fh is not None:
                    fxa = bass.AP(tensor=Xt.tensor,
                                  offset=Xt.offset + 4 * NI + x_off,
                                  ap=[Xt.ap[0], [NI, KT], [1, n_par], [0, 4]])
                    tf64 = tmp.tile([P, KT * nch], F32, name="t", tag="tf64",
                                    bufs=1)
                    nc.vector.tensor_add(
                        tf64[:].rearrange("p (m n g) -> p m n g", m=KT, g=4),
                        fh[:].rearrange("p (m n g) -> p m n g", m=KT, g=4),
                        fxa)
                    for m in range(KT):
                        nc.scalar.activation(F[:, m * nch:(m + 1) * nch],
                                             tf64[:, m * nch:(m + 1) * nch],
                                             AF.Sigmoid, bias=bf_s[:, m:m + 1])
                for m in range(KT if fh is None else 0):
                    p_f = ps.tile([P, MEM], F32, name="t", tag="psA", bufs=4)
                    for k in range(KT):
                        nc.tensor.matmul(
                            p_f[:, :nch], wf_s[k][:, m * P:(m + 1) * P],
                            Hc[:, k * nch:(k + 1) * nch],
                            start=(k == 0), stop=False, skip_group_check=True,
                        )
                    fx = bass.AP(tensor=Xt.tensor,
                                 offset=Xt.offset + (4 + m) * NI + x_off,
                                 ap=[Xt.ap[0], [1, n_par], [0, 4]])
                    nc.tensor.matmul(p_f[:, :nch], id_s[:], fx,
                                     start=False, stop=True,
                                     skip_group_check=True)
                    nc.scalar.activation(F[:, m * nch:(m + 1) * nch],
                                         p_f[:, :nch],
                                         AF.Sigmoid, bias=bf_s[:, m:m + 1])
                if mid_hook is not None:
                    mid_hook()
                # f*cc and its group-of-4 sum, pipelined per m-chunk so the
                # gpsimd mul and vector reduce overlap the next sigmoid
                FCC = tmp.tile([P, KT * nch], F16, name="t", tag="FCC")
                FCCS = tmp.tile([P, KT * n_par], F32, name="t", tag="FS")
                for m in range(KT):
                    cs = slice(m * nch, (m + 1) * nch)
                    nc.gpsimd.tensor_mul(FCC[:, cs], F[:, cs], Cc[:, cs])
                    nc.vector.tensor_reduce(
                        FCCS[:, m * n_par:(m + 1) * n_par].rearrange(
                            "p (o n) -> p o n", o=1),
                        FCC[:, cs].rearrange("p (o n g) -> p o n g", o=1, g=4),
                        axis=AX.X, op=ALU.add,
                    )
                # iou = Ws.T @ chs into one mono psum [128, 12*n_par], block
                # order i, u, o with activations fired per finished block: IU
                # and c need only i,u; the o gate is consumed last (for h).
                p_b = ps.tile([P, 12 * P], F32, name="t", tag="psB", bufs=1)
                GG = tmp.tile([P, 12 * n_par], F16, name="t", tag="GG")
                for mc in [0, 1, 2, 3, 8, 9, 10, 11, 4, 5, 6, 7]:
                    xt_mc = mc if mc < 4 else mc + 4
                    for k in range(KT):
                        nc.tensor.matmul(
                            p_b[:, mc * n_par:(mc + 1) * n_par],
                            ws_s[k][:, mc * P:(mc + 1) * P],
                            CHS[:, k * n_par:(k + 1) * n_par],
                            start=(k == 0), stop=False, skip_group_check=True,
                        )
                    xv = bass.AP(tensor=Xt.tensor,
                                 offset=Xt.offset + xt_mc * NI + x_off,
                                 ap=[Xt.ap[0], [1, n_par]])
                    nc.tensor.matmul(
                        p_b[:, mc * n_par:(mc + 1) * n_par],
                        id_s[:], xv, start=False, stop=True,
                        skip_group_check=True)
                    if mc == 3:
                        nc.scalar.activation(GG[:, :4 * n_par],
                                             p_b[:, :4 * n_par], AF.Sigmoid)
                    elif mc == 11:
                        nc.scalar.activation(GG[:, 8 * n_par:12 * n_par],
                                             p_b[:, 8 * n_par:12 * n_par],
                                             AF.Tanh)
                    elif mc == 7:
                        nc.scalar.activation(GG[:, 4 * n_par:8 * n_par],
                                             p_b[:, 4 * n_par:8 * n_par],
                                             AF.Sigmoid)
                IU = tmp.tile([P, KT * n_par], F32, name="t", tag="IU")
                nc.gpsimd.tensor_mul(IU[:], GG[:, :4 * n_par],
                                     GG[:, 8 * n_par:12 * n_par])
                Cp = state.tile([P, KT * n_par], F32, name="t", tag=f"C{hname}")
                nc.gpsimd.tensor_add(Cp[:], IU[:], FCCS[:])
                TH = tmp.tile([P, KT * n_par], F32, name="t", tag="TH")
                nc.scalar.activation(TH[:], Cp[:], AF.Tanh)
                Hp = state.tile([P, KT * n_par], h_dtype, name="t", tag=f"H{hname}")
                nc.gpsimd.tensor_mul(Hp[:], GG[:, 4 * n_par:8 * n_par], TH[:])
                return Hp, Cp

            H2, C2 = level_step(NL2, OFF2, H3, C3, "L2")
            H1, C1 = level_step(NL1, OFF1, H2, C2, "L1")
            H0, C0 = level_step(NL0, OFF0, H1, C1, "L0")

            # ---- gather the 64 subtree roots (h, c, Wf.T@h) to every core
            # contrib (f16) row p: [h f16 x32 | fh f16 x32 | c f32-as-2xf16].
            nc.sync.dma_start(contrib_d[:, 0:32], H0[:])
            nc.sync.dma_start(contrib_d[:, 64:128], C0[:].bitcast(F16))
            FH0 = tmp.tile([P, KT * NL0], F16, name="t", tag="FH0", bufs=1)
            for m in range(KT):
                p_h = ps.tile([P, MEM], F32, name="t", tag="psA", bufs=4)
                for k in range(KT):
                    nc.tensor.matmul(
                        p_h[:, :NL0], wf_s[k][:, m * P:(m + 1) * P],
                        H0[:, k * NL0:(k + 1) * NL0],
                        start=(k == 0), stop=(k == KT - 1),
                        skip_group_check=True,
                    )
                nc.vector.tensor_copy(FH0[:, m * NL0:(m + 1) * NL0],
                                      p_h[:, :NL0])
            nc.sync.dma_start(contrib_d[:, 32:64], FH0[:])
            nc.gpsimd.collective_compute(
                "AllGather", ALU.bypass,
                replica_groups=[list(range(NCORES))],
                ins=[contrib_d[:]],
                outs=[gath_d[:]],
            )
            # one contiguous load (256B runs), then engine-permute columns to
            # subtree order k = 8*s + c (T2 child column k).
            GR = state.tile([P, 2 * KT * 128], F16, name="t", tag="GR")
            H64 = state.tile([P, KT * 64], F16, name="t", tag="H64")
            C64 = state.tile([P, KT * 64], F32, name="t", tag="C64")
            FH64 = state.tile([P, KT * 64], F32, name="t", tag="FH64")
            nc.sync.dma_start(
                GR[:].rearrange("p (c j) -> p c j", c=NCORES),
                gath_d[:].rearrange("(c p) j -> p c j", c=NCORES))
            grf = GR[:].bitcast(F32)  # [128, 512]: c block at f32 col 32+
            perm_in = lambda base, off, cs: bass.AP(
                tensor=base.tensor, offset=base.offset + off,
                ap=[base.ap[0], [NL0, KT], [1, NL0], [cs, NCORES]])
            perm_out = lambda t: bass.AP(
                tensor=t.tensor, offset=t.offset,
                ap=[t.ap[0], [64, KT], [NL0, NL0], [1, NCORES]])
            nc.gpsimd.tensor_copy(perm_out(FH64[:]), perm_in(GR[:], 32, 128))
            nc.vector.tensor_copy(perm_out(H64[:]), perm_in(GR[:], 0, 128))
            nc.gpsimd.tensor_copy(perm_out(C64[:]), perm_in(grf, 32, 64))

            HT2, CT2 = level_step(16, OFFT2, H64, C64, "T2", fh=FH64)
            HT1, CT1 = level_step(4, OFFT1, HT2, CT2, "T1")
            HT0, _ = level_step(1, OFFT0, HT1, CT1, "T0", h_dtype=F32)
            nc.sync.dma_start(out_d[:], HT0[:])

    nc.compile()
    return nc


_NC_CACHE = None


def kernel(inputs, Wx, bx, Ws, bs, Wf, bf, children):
    global LAST_RESULT, _NC_CACHE
    inputs = np.asarray(inputs, np.float32)
    Wx = np.asarray(Wx, np.float32)
    bx = np.asarray(bx, np.float32)
    Ws = np.asarray(Ws, np.float32)
    bs = np.asarray(bs, np.float32)
    Wf = np.asarray(Wf, np.float32)
    bf = np.asarray(bf, np.float32)

    Wx_b = Wx.astype(np.float16)
    Ws_b = Ws.astype(np.float16)
    Wf_b = Wf.astype(np.float16)
    bxT = bx.reshape(16, P).T          # [128, 16] col mc
    bsT = bs.reshape(12, P).T          # [128, 12] col: i0..3 o0..3 u0..3
    bfT = np.ascontiguousarray(bf.reshape(4, P).T)
    # bxc: bx with bs folded into the i/o/u blocks (internal-X bias)
    bxc = bxT.copy()
    bxc[:, 0:4] += bsT[:, 0:4]
    bxc[:, 8:12] += bsT[:, 4:8]
    bxc[:, 12:16] += bsT[:, 8:12]
    bxc = np.ascontiguousarray(bxc)
    # bxs: leaf-gate bias (i,o,u blocks of bx + bs)
    bxs = np.concatenate(
        [bxT[:, 0:4] + bsT[:, 0:4], bxT[:, 8:12] + bsT[:, 4:8],
         bxT[:, 12:16] + bsT[:, 8:12]], axis=1)
    bxs = np.ascontiguousarray(bxs)

    swf = np.ascontiguousarray(np.concatenate([Ws_b, Wf_b], axis=1))
    in_maps = []
    for c in range(NCORES):
        heaps = _core_heaps(c)
        valid = heaps >= 0
        M = np.zeros((NX, IN_DIM), np.float32)
        M[valid] = inputs[N - 1 - heaps[valid]]
        xin = np.ascontiguousarray(M.T)
        mrow = valid[NI:].astype(np.float32)
        cmask = np.ascontiguousarray(np.tile(mrow[None, :], (P, 1)))
        xwx = np.concatenate([xin.astype(np.float16), Wx_b], axis=1)
        if FP8_A:
            import ml_dtypes
            xwx = xwx.astype(np.float32).astype(ml_dtypes.float8_e4m3)
        bp = np.concatenate([bxc, bxs, bfT, cmask,
                             np.eye(P, dtype=np.float32)], axis=1)
        in_maps.append({
            "xwx": np.ascontiguousarray(xwx), "swf": swf,
            "bp": np.ascontiguousarray(bp),
        })

    if _NC_CACHE is None:
        _NC_CACHE = _build_program()
    nc = _NC_CACHE

    res = run_bass_kernel_spmd(
        nc, in_maps, list(range(NCORES)),
        trace=bool(os.environ.get("BASS_TRACE")),
    )
    LAST_RESULT = res

    out = np.asarray(res.results[0]["out"])  # [128, 4]; h[m*128+p] = out[p, m]
    return np.ascontiguousarray(out.T.reshape(1, MEM))
